# revision 27
# baseline (speedup 1.0000x reference)
"""Trainium2 Bass kernel for nn_CustomLoss_90434831384972 (Hungarian matching loss).

Contract: kernel(**inputs) takes FULL inputs (batch_prediction [32,128,256] f32,
batch_groundtruth [32,128,256] f32) and returns the FULL output (loss, shape [1] f32).

Strategy (data-parallel over batch across 8 NeuronCores, 4 batches/core):
  device: pg[b,n,m] = p[b,n,:] . g[b,m,:]  -- the O(N*M*D) part. Inputs are
          sharded per-core and laid out d-major on the host so the
          TensorEngine consumes them directly with no on-chip transposes;
          operands are bf16 (fp32 PSUM accumulation), which halves load
          traffic and runs the PE at full rate. Measured end-to-end loss
          rel-err vs the fp32 reference: 1.1e-6.
  host:   cost = (p2[:,None] + g2[None,:] - 2*pg)/D  (rank-1 terms, O(N*M)),
          Hungarian assignment per batch (sequential and host-bound exactly
          as in the reference), matched-cost mean -> loss.
"""

import ml_dtypes
import numpy as np

import concourse.bass as bass
import concourse.tile as tile
from concourse import mybir
from concourse.bass_utils import run_bass_kernel_spmd

N_CORES = 8
B, N, M, D = 32, 128, 128, 256
BPC = B // N_CORES  # batches per core
F32 = mybir.dt.float32
WEIGHT = 1.0


# --------------------------------------------------------------------------
# Workaround: this walrus build encodes at most ONE sem-wait per instruction.
# Tile's sem-assignment can pile several waits onto one instruction (notably
# the kernel-tail Drain). Keep the last wait, hoist the rest onto same-engine
# NoOps inserted immediately before (same engine stream => same semantics).
def _split_multi_waits(nc):
    for fn in nc.m.functions:
        for bb in fn.blocks:
            new_instrs = []
            for inst in bb.instructions:
                si = inst.sync_info
                if si is not None and si.on_wait and len(si.on_wait) > 1:
                    extra, keep = si.on_wait[:-1], si.on_wait[-1:]
                    for k, w in enumerate(extra):
                        nop = mybir.InstNoOp(
                            name=f"{inst.name}_wsplit{k}", ins=[], outs=[]
                        )
                        nop.engine = inst.engine
                        nop.sync_info = mybir.SyncInfo(on_wait=[w], on_update=[])
                        new_instrs.append(nop)
                    si.on_wait = keep
                new_instrs.append(inst)
            bb.instructions = new_instrs


KC = D // 128  # contraction chunks per batch
XW = KC * (N + M)  # combined per-batch row width (pT cols then gT cols)


def _strip_init_barrier(nc):
    """Drop the module-init const memsets + all-engine barrier from the 'main'
    block. Our kernel never reads the const APs (no activations/memsets), and
    all cross-engine deps flow through Tile-assigned semaphores, so engines
    can branch straight into the kernel body."""
    for fn in nc.m.functions:
        for bb in fn.blocks:
            if bb.name != "main":
                continue
            keep = []
            for inst in bb.instructions:
                nm = type(inst).__name__
                if nm == "InstMemset":
                    continue
                if nm in ("InstEventSemaphore", "InstDrain"):
                    continue
                keep.append(inst)
            bb.instructions = keep


def build_pg_kernel(
    load_split=(1, 1, 1, 1),  # batches per load DMA (combined pT+gT)
    store_split=(1, 1, 1, 1),  # batches per store DMA
    load_eng=("sync", "scalar", "sync", "scalar"),  # engine per load DMA
    store_eng=("scalar", "sync", "scalar", "sync"),  # engine per store DMA
    copy_eng=("vector", "vector", "vector", "vector"),  # psum->sbuf copy engine
    in_dt=F32,
    strip_init=False,
):
    """Per-core SPMD program.

    Input (pre-transposed, interleaved on host):
      io [BPC, 128, XW] with io[b, p, c*N+n] = p[b, n, c*128+p]
                         and io[b, p, KC*N + c*M+m] = g[b, m, c*128+p]
    Output:
      pg [N, BPC, M] f32 with pg[n, b, m] = sum_d p[b,n,d] g[b,m,d]
    """
    assert sum(load_split) == BPC and sum(store_split) == BPC

    nc = bass.Bass()
    io_in = nc.dram_tensor("io", [BPC, 128, XW], in_dt, kind="ExternalInput")
    pg = nc.dram_tensor("pg", [N, BPC, M], F32, kind="ExternalOutput")

    def eng(name):
        return {"sync": nc.sync, "scalar": nc.scalar, "gpsimd": nc.gpsimd,
                "vector": nc.vector}[name]

    use_act_copy = any(c.startswith("split") or c.startswith("scalar") for c in copy_eng)
    with tile.TileContext(nc) as tc:
        with (
            tc.tile_pool(name="io", bufs=2) as iop,
            tc.tile_pool(name="out", bufs=2) as outp,
            tc.tile_pool(name="pacc", bufs=2, space="PSUM") as pacc,
        ):
            if use_act_copy:
                # warm the ACT Copy table during load dead-time
                warm = outp.tile([1, 1], F32, tag="warm")
                nc.scalar.copy(warm[:], warm[:])
            tiles = {}  # batch -> (tile, local index)
            b0 = 0
            for li, nb in enumerate(load_split):
                t = iop.tile([128, nb, XW], in_dt, tag=f"io{li}_{in_dt}")
                if nb == 1:
                    eng(load_eng[li]).dma_start(t[:, 0, :], io_in[b0])
                else:
                    eng(load_eng[li]).dma_start(
                        t[:], io_in[b0 : b0 + nb].rearrange("b p x -> p b x")
                    )
                for k in range(nb):
                    tiles[b0 + k] = (t, k)
                b0 += nb

            b0 = 0
            for si, nb in enumerate(store_split):
                acc = pacc.tile([N, nb, M], F32, tag=f"acc{si % 2}")
                out_sb = outp.tile([N, nb, M], F32, tag=f"out{si % 2}")
                for k in range(nb):
                    t, tk = tiles[b0 + k]
                    for c in range(KC):
                        nc.tensor.matmul(
                            acc[:, k, :],
                            t[:, tk, c * N : (c + 1) * N],
                            t[:, tk, KC * N + c * M : KC * N + (c + 1) * M],
                            start=(c == 0),
                            stop=(c == KC - 1),
                        )
                    if copy_eng[si].endswith("_b"):  # per-batch copy granularity
                        e = copy_eng[si][:-2]
                        if e == "vector":
                            nc.vector.tensor_copy(out_sb[:, k, :], acc[:, k, :])
                        else:
                            nc.scalar.copy(out_sb[:, k, :], acc[:, k, :])
                    elif copy_eng[si] == "split":  # parallel DVE+ACT per batch
                        if k % 2 == 0:
                            nc.scalar.copy(out_sb[:, k, :], acc[:, k, :])
                        else:
                            nc.vector.tensor_copy(out_sb[:, k, :], acc[:, k, :])
                if not (copy_eng[si].endswith("_b") or copy_eng[si] == "split"):
                    if copy_eng[si] == "vector":
                        nc.vector.tensor_copy(out_sb[:], acc[:])
                    else:
                        nc.scalar.copy(out_sb[:], acc[:])
                eng(store_eng[si]).dma_start(pg[:, b0 : b0 + nb, :], out_sb[:])
                b0 += nb

    if strip_init:
        _strip_init_barrier(nc)

    _split_multi_waits(nc)
    return nc


BEST_CONFIG = dict(
    load_split=(2, 2),
    store_split=(2, 2),
    load_eng=("sync", "gpsimd"),
    store_eng=("scalar", "sync"),
    copy_eng=("vector", "vector"),
    in_dt=mybir.dt.bfloat16,
    strip_init=True,
)
HOST_DT = ml_dtypes.bfloat16

_NC_CACHE = None


def _get_nc():
    global _NC_CACHE
    if _NC_CACHE is None:
        _NC_CACHE = build_pg_kernel(**BEST_CONFIG)
    return _NC_CACHE


# --------------------------------------------------------------------------
# Host-side Hungarian (Jonker-Volgenant potentials form), matches
# scipy.linear_sum_assignment. Sequential and host-bound by nature.
def _hungarian(cost):
    cost = np.asarray(cost, dtype=np.float64)
    n, m = cost.shape
    u = np.zeros(n + 1)
    v = np.zeros(m + 1)
    p = np.zeros(m + 1, dtype=np.int64)
    way = np.zeros(m + 1, dtype=np.int64)
    for i in range(1, n + 1):
        p[0] = i
        j0 = 0
        minv = np.full(m + 1, np.inf)
        used = np.zeros(m + 1, dtype=bool)
        while True:
            used[j0] = True
            i0 = p[j0]
            free = ~used[1:]
            cur = cost[i0 - 1, :] - u[i0] - v[1:]
            upd = free & (cur < minv[1:])
            minv[1:][upd] = cur[upd]
            way[1:][upd] = j0
            masked = np.where(free, minv[1:], np.inf)
            j1 = int(np.argmin(masked)) + 1
            delta = masked[j1 - 1]
            u[p[used]] += delta
            v[used] -= delta
            minv[1:][free] -= delta
            j0 = j1
            if p[j0] == 0:
                break
        while j0:
            j1 = way[j0]
            p[j0] = p[j1]
            j0 = j1
    col_of_row = np.zeros(n, dtype=np.int64)
    for j in range(1, m + 1):
        if p[j] > 0:
            col_of_row[p[j] - 1] = j - 1
    return col_of_row


def _chunked_transpose(x):
    """[B, N, D] -> [B, 128, KC*N] with out[b, p, c*N+n] = x[b, n, c*128+p]."""
    # x.T per batch: [B, D, N] -> view as [B, KC, 128, N] -> [B, 128, KC, N]
    xt = x.transpose(0, 2, 1).reshape(x.shape[0], KC, 128, x.shape[1])
    return xt.transpose(0, 2, 1, 3).reshape(x.shape[0], 128, KC * x.shape[1])


def host_pack(p_full, g_full, dtype=np.float32):
    """Build the combined device input io [B, 128, XW]."""
    io = np.concatenate(
        [_chunked_transpose(p_full), _chunked_transpose(g_full)], axis=2
    )
    return np.ascontiguousarray(io.astype(dtype, copy=False))


def run_device_pg(p_full, g_full):
    """Shard over batch (d-major layout), run SPMD on 8 cores, gather pg."""
    nc = _get_nc()
    io = host_pack(p_full, g_full, dtype=HOST_DT)
    in_maps = [{"io": io[c * BPC : (c + 1) * BPC]} for c in range(N_CORES)]
    try:
        res = run_bass_kernel_spmd(nc, in_maps, list(range(N_CORES)))
    except ModuleNotFoundError:
        # BASS_TRACE set but the axon NTFF hook module isn't installed —
        # retry with tracing hard-disabled.
        import os

        os.environ["BASS_NEVER_TRACE"] = "1"
        res = run_bass_kernel_spmd(nc, in_maps, list(range(N_CORES)))
    # per-core result is pg_t[n, b, m] -> [b, n, m]
    return np.concatenate(
        [res.results[c]["pg"].transpose(1, 0, 2) for c in range(N_CORES)], axis=0
    )


def kernel(batch_prediction, batch_groundtruth):
    p_full = np.asarray(batch_prediction, dtype=np.float32)
    g_full = np.asarray(batch_groundtruth, dtype=np.float32)
    pg = run_device_pg(p_full, g_full)  # [B, N, M] f32 (bf16 products, f32 accum)

    # rank-1 terms + scaling on host (fp64), then Hungarian + matched mean
    p64 = p_full.astype(np.float64)
    g64 = g_full.astype(np.float64)
    p2 = np.einsum("bnd,bnd->bn", p64, p64)
    g2 = np.einsum("bmd,bmd->bm", g64, g64)
    cost = (p2[:, :, None] + g2[:, None, :] - 2.0 * pg.astype(np.float64)) / D

    total = 0.0
    rows = np.arange(N)
    for b in range(B):
        cols = _hungarian(cost[b])
        total += float(cost[b, rows, cols].mean())
    loss = np.float32(total / B * WEIGHT)
    return np.asarray([loss], dtype=np.float32)


# revision 36
# speedup vs baseline: 1.0479x; 1.0479x over previous
"""Trainium2 Bass kernel for nn_CustomLoss_90434831384972 (Hungarian matching loss).

Contract: kernel(**inputs) takes FULL inputs (batch_prediction [32,128,256] f32,
batch_groundtruth [32,128,256] f32) and returns the FULL output (loss, shape [1] f32).

Strategy (data-parallel over batch across 8 NeuronCores, 4 batches/core):
  device: pg[b,n,m] = p[b,n,:] . g[b,m,:]  -- the O(N*M*D) part. Inputs are
          sharded per-core and laid out d-major on the host so the
          TensorEngine consumes them directly with no on-chip transposes;
          operands are bf16 (fp32 PSUM accumulation), which halves load
          traffic and runs the PE at full rate. Measured end-to-end loss
          rel-err vs the fp32 reference: 1.1e-6.
  host:   cost = (p2[:,None] + g2[None,:] - 2*pg)/D  (rank-1 terms, O(N*M)),
          Hungarian assignment per batch (sequential and host-bound exactly
          as in the reference), matched-cost mean -> loss.
"""

import ml_dtypes
import numpy as np

import concourse.bass as bass
import concourse.tile as tile
from concourse import mybir
from concourse.bass_utils import run_bass_kernel_spmd

N_CORES = 8
B, N, M, D = 32, 128, 128, 256
BPC = B // N_CORES  # batches per core
F32 = mybir.dt.float32
WEIGHT = 1.0


# --------------------------------------------------------------------------
# Workaround: this walrus build encodes at most ONE sem-wait per instruction.
# Tile's sem-assignment can pile several waits onto one instruction (notably
# the kernel-tail Drain). Keep the last wait, hoist the rest onto same-engine
# NoOps inserted immediately before (same engine stream => same semantics).
def _split_multi_waits(nc):
    for fn in nc.m.functions:
        for bb in fn.blocks:
            new_instrs = []
            for inst in bb.instructions:
                si = inst.sync_info
                if si is not None and si.on_wait and len(si.on_wait) > 1:
                    extra, keep = si.on_wait[:-1], si.on_wait[-1:]
                    for k, w in enumerate(extra):
                        nop = mybir.InstNoOp(
                            name=f"{inst.name}_wsplit{k}", ins=[], outs=[]
                        )
                        nop.engine = inst.engine
                        nop.sync_info = mybir.SyncInfo(on_wait=[w], on_update=[])
                        new_instrs.append(nop)
                    si.on_wait = keep
                new_instrs.append(inst)
            bb.instructions = new_instrs


KC = D // 128  # contraction chunks per batch
XW = KC * (N + M)  # combined per-batch row width (pT cols then gT cols)


def _strip_init_barrier(nc, strip_sp_regs=False):
    """Drop the module-init const memsets + all-engine barrier from the 'main'
    block. Our kernel never reads the const APs (no activations/memsets), and
    all cross-engine deps flow through Tile-assigned semaphores, so engines
    can branch straight into the kernel body.

    strip_sp_regs additionally drops SP's preamble RegisterMoves (zero +
    bounds-check regs) — SP only issues static-AP DMAs/drains, which never
    read those registers, and SP's first DMA is on the critical path."""
    for fn in nc.m.functions:
        for bb in fn.blocks:
            if bb.name != "main":
                continue
            keep = []
            for inst in bb.instructions:
                nm = type(inst).__name__
                if nm == "InstMemset":
                    continue
                if nm in ("InstEventSemaphore", "InstDrain"):
                    continue
                if (
                    strip_sp_regs
                    and nm == "InstRegisterMove"
                    and inst.engine in (mybir.EngineType.SP, mybir.EngineType.Pool)
                ):
                    continue
                keep.append(inst)
            bb.instructions = keep


def build_pg_kernel(
    load_split=(1, 1, 1, 1),  # batches per load DMA (combined pT+gT)
    store_split=(1, 1, 1, 1),  # batches per store DMA
    load_eng=("sync", "scalar", "sync", "scalar"),  # engine per load DMA
    store_eng=("scalar", "sync", "scalar", "sync"),  # engine per store DMA
    copy_eng=("vector", "vector", "vector", "vector"),  # psum->sbuf copy engine
    in_dt=F32,
    strip_init=False,
    pin_pe_order=False,
    strip_sp_regs=False,
):
    """Per-core SPMD program.

    Input (pre-transposed, interleaved on host):
      io [BPC, 128, XW] with io[b, p, c*N+n] = p[b, n, c*128+p]
                         and io[b, p, KC*N + c*M+m] = g[b, m, c*128+p]
    Output:
      pg [N, BPC, M] f32 with pg[n, b, m] = sum_d p[b,n,d] g[b,m,d]
    """
    assert sum(load_split) == BPC and sum(store_split) == BPC

    nc = bass.Bass()
    io_in = nc.dram_tensor("io", [BPC, 128, XW], in_dt, kind="ExternalInput")
    pg = nc.dram_tensor("pg", [N, BPC, M], F32, kind="ExternalOutput")

    def eng(name):
        return {"sync": nc.sync, "scalar": nc.scalar, "gpsimd": nc.gpsimd,
                "vector": nc.vector}[name]

    use_act_copy = any(c.startswith("split") or c.startswith("scalar") for c in copy_eng)
    with tile.TileContext(nc) as tc:
        with (
            tc.tile_pool(name="io", bufs=2) as iop,
            tc.tile_pool(name="out", bufs=2) as outp,
            tc.tile_pool(name="pacc", bufs=2, space="PSUM") as pacc,
        ):
            if use_act_copy:
                # warm the ACT Copy table during load dead-time
                warm = outp.tile([1, 1], F32, tag="warm")
                nc.scalar.copy(warm[:], warm[:])
            tiles = {}  # batch -> (tile, local index)
            b0 = 0
            for li, nb in enumerate(load_split):
                t = iop.tile([128, nb, XW], in_dt, tag=f"io{li}_{in_dt}")
                if nb == 1:
                    eng(load_eng[li]).dma_start(t[:, 0, :], io_in[b0])
                else:
                    eng(load_eng[li]).dma_start(
                        t[:], io_in[b0 : b0 + nb].rearrange("b p x -> p b x")
                    )
                for k in range(nb):
                    tiles[b0 + k] = (t, k)
                b0 += nb

            prev_mm = None
            b0 = 0
            for si, nb in enumerate(store_split):
                per_batch = copy_eng[si].endswith("_b") or copy_eng[si] == "split"
                if not per_batch:
                    acc = pacc.tile([N, nb, M], F32, tag=f"acc{si % 2}")
                out_sb = outp.tile([N, nb, M], F32, tag=f"out{si % 2}")
                for k in range(nb):
                    if per_batch:
                        # own PSUM tile per batch: avoids the same-bank
                        # PE-write vs copy-read serialization within a pair
                        acc_k = pacc.tile([N, 1, M], F32, tag=f"accb{b0 + k}")
                        dst = acc_k[:, 0, :]
                    else:
                        dst = acc[:, k, :]
                    t, tk = tiles[b0 + k]
                    for c in range(KC):
                        mm = nc.tensor.matmul(
                            dst,
                            t[:, tk, c * N : (c + 1) * N],
                            t[:, tk, KC * N + c * M : KC * N + (c + 1) * M],
                            start=(c == 0),
                            stop=(c == KC - 1),
                        )
                        if pin_pe_order and prev_mm is not None:
                            # keep PE instruction order = batch order so PE
                            # never idles waiting on a later batch's load
                            tile.add_dep_helper(
                                mm.ins, prev_mm.ins, sync=False,
                                reason="pin PE batch order",
                            )
                        prev_mm = mm
                    if per_batch:
                        src = acc_k[:, 0, :]
                        if copy_eng[si] == "vector_b":
                            nc.vector.tensor_copy(out_sb[:, k, :], src)
                        elif copy_eng[si] == "scalar_b":
                            nc.scalar.copy(out_sb[:, k, :], src)
                        elif k % 2 == 0:  # split
                            nc.scalar.copy(out_sb[:, k, :], src)
                        else:
                            nc.vector.tensor_copy(out_sb[:, k, :], src)
                if not per_batch:
                    if copy_eng[si] == "vector":
                        nc.vector.tensor_copy(out_sb[:], acc[:])
                    else:
                        nc.scalar.copy(out_sb[:], acc[:])
                eng(store_eng[si]).dma_start(pg[:, b0 : b0 + nb, :], out_sb[:])
                b0 += nb

    if strip_init:
        _strip_init_barrier(nc, strip_sp_regs=strip_sp_regs)

    _split_multi_waits(nc)
    return nc


BEST_CONFIG = dict(
    load_split=(2, 2),
    store_split=(2, 2),
    load_eng=("sync", "gpsimd"),
    store_eng=("scalar", "sync"),
    copy_eng=("split", "split"),
    in_dt=mybir.dt.bfloat16,
    strip_init=True,
    pin_pe_order=True,
    strip_sp_regs=True,
)
HOST_DT = ml_dtypes.bfloat16

_NC_CACHE = None


def _get_nc():
    global _NC_CACHE
    if _NC_CACHE is None:
        _NC_CACHE = build_pg_kernel(**BEST_CONFIG)
    return _NC_CACHE


# --------------------------------------------------------------------------
# Host-side Hungarian (Jonker-Volgenant potentials form), matches
# scipy.linear_sum_assignment. Sequential and host-bound by nature.
def _hungarian(cost):
    cost = np.asarray(cost, dtype=np.float64)
    n, m = cost.shape
    u = np.zeros(n + 1)
    v = np.zeros(m + 1)
    p = np.zeros(m + 1, dtype=np.int64)
    way = np.zeros(m + 1, dtype=np.int64)
    for i in range(1, n + 1):
        p[0] = i
        j0 = 0
        minv = np.full(m + 1, np.inf)
        used = np.zeros(m + 1, dtype=bool)
        while True:
            used[j0] = True
            i0 = p[j0]
            free = ~used[1:]
            cur = cost[i0 - 1, :] - u[i0] - v[1:]
            upd = free & (cur < minv[1:])
            minv[1:][upd] = cur[upd]
            way[1:][upd] = j0
            masked = np.where(free, minv[1:], np.inf)
            j1 = int(np.argmin(masked)) + 1
            delta = masked[j1 - 1]
            u[p[used]] += delta
            v[used] -= delta
            minv[1:][free] -= delta
            j0 = j1
            if p[j0] == 0:
                break
        while j0:
            j1 = way[j0]
            p[j0] = p[j1]
            j0 = j1
    col_of_row = np.zeros(n, dtype=np.int64)
    for j in range(1, m + 1):
        if p[j] > 0:
            col_of_row[p[j] - 1] = j - 1
    return col_of_row


def _chunked_transpose(x):
    """[B, N, D] -> [B, 128, KC*N] with out[b, p, c*N+n] = x[b, n, c*128+p]."""
    # x.T per batch: [B, D, N] -> view as [B, KC, 128, N] -> [B, 128, KC, N]
    xt = x.transpose(0, 2, 1).reshape(x.shape[0], KC, 128, x.shape[1])
    return xt.transpose(0, 2, 1, 3).reshape(x.shape[0], 128, KC * x.shape[1])


def host_pack(p_full, g_full, dtype=np.float32):
    """Build the combined device input io [B, 128, XW]."""
    io = np.concatenate(
        [_chunked_transpose(p_full), _chunked_transpose(g_full)], axis=2
    )
    return np.ascontiguousarray(io.astype(dtype, copy=False))


def run_device_pg(p_full, g_full):
    """Shard over batch (d-major layout), run SPMD on 8 cores, gather pg."""
    nc = _get_nc()
    io = host_pack(p_full, g_full, dtype=HOST_DT)
    in_maps = [{"io": io[c * BPC : (c + 1) * BPC]} for c in range(N_CORES)]
    try:
        res = run_bass_kernel_spmd(nc, in_maps, list(range(N_CORES)))
    except ModuleNotFoundError:
        # BASS_TRACE set but the axon NTFF hook module isn't installed —
        # retry with tracing hard-disabled.
        import os

        os.environ["BASS_NEVER_TRACE"] = "1"
        res = run_bass_kernel_spmd(nc, in_maps, list(range(N_CORES)))
    # per-core result is pg_t[n, b, m] -> [b, n, m]
    return np.concatenate(
        [res.results[c]["pg"].transpose(1, 0, 2) for c in range(N_CORES)], axis=0
    )


def kernel(batch_prediction, batch_groundtruth):
    p_full = np.asarray(batch_prediction, dtype=np.float32)
    g_full = np.asarray(batch_groundtruth, dtype=np.float32)
    pg = run_device_pg(p_full, g_full)  # [B, N, M] f32 (bf16 products, f32 accum)

    # rank-1 terms + scaling on host (fp64), then Hungarian + matched mean
    p64 = p_full.astype(np.float64)
    g64 = g_full.astype(np.float64)
    p2 = np.einsum("bnd,bnd->bn", p64, p64)
    g2 = np.einsum("bmd,bmd->bm", g64, g64)
    cost = (p2[:, :, None] + g2[:, None, :] - 2.0 * pg.astype(np.float64)) / D

    total = 0.0
    rows = np.arange(N)
    for b in range(B):
        cols = _hungarian(cost[b])
        total += float(cost[b, rows, cols].mean())
    loss = np.float32(total / B * WEIGHT)
    return np.asarray([loss], dtype=np.float32)


# revision 38
# speedup vs baseline: 1.0852x; 1.0356x over previous
"""Trainium2 Bass kernel for nn_CustomLoss_90434831384972 (Hungarian matching loss).

Contract: kernel(**inputs) takes FULL inputs (batch_prediction [32,128,256] f32,
batch_groundtruth [32,128,256] f32) and returns the FULL output (loss, shape [1] f32).

Strategy (data-parallel over batch across 8 NeuronCores, 4 batches/core):
  device: pg[b,n,m] = p[b,n,:] . g[b,m,:]  -- the O(N*M*D) part. Inputs are
          sharded per-core and laid out d-major on the host so the
          TensorEngine consumes them directly with no on-chip transposes;
          operands are bf16 (fp32 PSUM accumulation), which halves load
          traffic and runs the PE at full rate. Measured end-to-end loss
          rel-err vs the fp32 reference: 1.1e-6.
  host:   cost = (p2[:,None] + g2[None,:] - 2*pg)/D  (rank-1 terms, O(N*M)),
          Hungarian assignment per batch (sequential and host-bound exactly
          as in the reference), matched-cost mean -> loss.
"""

import ml_dtypes
import numpy as np

import concourse.bass as bass
import concourse.tile as tile
from concourse import mybir
from concourse.bass_utils import run_bass_kernel_spmd

N_CORES = 8
B, N, M, D = 32, 128, 128, 256
BPC = B // N_CORES  # batches per core
F32 = mybir.dt.float32
WEIGHT = 1.0


# --------------------------------------------------------------------------
# Workaround: this walrus build encodes at most ONE sem-wait per instruction.
# Tile's sem-assignment can pile several waits onto one instruction (notably
# the kernel-tail Drain). Keep the last wait, hoist the rest onto same-engine
# NoOps inserted immediately before (same engine stream => same semantics).
def _split_multi_waits(nc):
    for fn in nc.m.functions:
        for bb in fn.blocks:
            new_instrs = []
            for inst in bb.instructions:
                si = inst.sync_info
                if si is not None and si.on_wait and len(si.on_wait) > 1:
                    extra, keep = si.on_wait[:-1], si.on_wait[-1:]
                    for k, w in enumerate(extra):
                        nop = mybir.InstNoOp(
                            name=f"{inst.name}_wsplit{k}", ins=[], outs=[]
                        )
                        nop.engine = inst.engine
                        nop.sync_info = mybir.SyncInfo(on_wait=[w], on_update=[])
                        new_instrs.append(nop)
                    si.on_wait = keep
                new_instrs.append(inst)
            bb.instructions = new_instrs


KC = D // 128  # contraction chunks per batch
XW = KC * (N + M)  # combined per-batch row width (pT cols then gT cols)


def _strip_init_barrier(nc, strip_sp_regs=False):
    """Drop the module-init const memsets + all-engine barrier from the 'main'
    block. Our kernel never reads the const APs (no activations/memsets), and
    all cross-engine deps flow through Tile-assigned semaphores, so engines
    can branch straight into the kernel body.

    strip_sp_regs additionally drops SP's preamble RegisterMoves (zero +
    bounds-check regs) — SP only issues static-AP DMAs/drains, which never
    read those registers, and SP's first DMA is on the critical path."""
    for fn in nc.m.functions:
        for bb in fn.blocks:
            if bb.name != "main":
                continue
            keep = []
            for inst in bb.instructions:
                nm = type(inst).__name__
                if nm == "InstMemset":
                    continue
                if nm in ("InstEventSemaphore", "InstDrain"):
                    continue
                if (
                    strip_sp_regs
                    and nm == "InstRegisterMove"
                    and inst.engine in (mybir.EngineType.SP, mybir.EngineType.Pool)
                ):
                    continue
                keep.append(inst)
            bb.instructions = keep


def _strip_tail_barrier2(nc):
    """Drop the post-sem-clear all-engine barrier at kernel tail. Engines halt
    independently right after; the NEFF execute boundary (runtime waits for
    every engine to halt before any re-execution) provides the same ordering
    the second barrier did. The pre-clear barrier and the global drain stay."""
    for fn in nc.m.functions:
        for bb in fn.blocks:
            if not bb.name.endswith("_end"):
                continue
            last_isa = None
            for i, inst in enumerate(bb.instructions):
                if type(inst).__name__ == "InstISA":
                    last_isa = i
            if last_isa is not None:
                bb.instructions = bb.instructions[: last_isa + 1]


def build_pg_kernel(
    load_split=(1, 1, 1, 1),  # batches per load DMA (combined pT+gT)
    store_split=(1, 1, 1, 1),  # batches per store DMA
    load_eng=("sync", "scalar", "sync", "scalar"),  # engine per load DMA
    store_eng=("scalar", "sync", "scalar", "sync"),  # engine per store DMA
    copy_eng=("vector", "vector", "vector", "vector"),  # psum->sbuf copy engine
    in_dt=F32,
    strip_init=False,
    pin_pe_order=False,
    strip_sp_regs=False,
    strip_tail2=False,
):
    """Per-core SPMD program.

    Input (pre-transposed, interleaved on host):
      io [BPC, 128, XW] with io[b, p, c*N+n] = p[b, n, c*128+p]
                         and io[b, p, KC*N + c*M+m] = g[b, m, c*128+p]
    Output:
      pg [N, BPC, M] f32 with pg[n, b, m] = sum_d p[b,n,d] g[b,m,d]
    """
    assert sum(load_split) == BPC and sum(store_split) == BPC

    nc = bass.Bass()
    io_in = nc.dram_tensor("io", [BPC, 128, XW], in_dt, kind="ExternalInput")
    pg = nc.dram_tensor("pg", [N, BPC, M], F32, kind="ExternalOutput")

    def eng(name):
        return {"sync": nc.sync, "scalar": nc.scalar, "gpsimd": nc.gpsimd,
                "vector": nc.vector}[name]

    use_act_copy = any(c.startswith("split") or c.startswith("scalar") for c in copy_eng)
    with tile.TileContext(nc) as tc:
        with (
            tc.tile_pool(name="io", bufs=2) as iop,
            tc.tile_pool(name="out", bufs=2) as outp,
            tc.tile_pool(name="pacc", bufs=2, space="PSUM") as pacc,
        ):
            if use_act_copy:
                # warm the ACT Copy table during load dead-time
                warm = outp.tile([1, 1], F32, tag="warm")
                nc.scalar.copy(warm[:], warm[:])
            tiles = {}  # batch -> (tile, local index)
            b0 = 0
            for li, nb in enumerate(load_split):
                t = iop.tile([128, nb, XW], in_dt, tag=f"io{li}_{in_dt}")
                if nb == 1:
                    eng(load_eng[li]).dma_start(t[:, 0, :], io_in[b0])
                else:
                    eng(load_eng[li]).dma_start(
                        t[:], io_in[b0 : b0 + nb].rearrange("b p x -> p b x")
                    )
                for k in range(nb):
                    tiles[b0 + k] = (t, k)
                b0 += nb

            prev_mm = None
            b0 = 0
            for si, nb in enumerate(store_split):
                per_batch = copy_eng[si].endswith("_b") or copy_eng[si] == "split"
                if not per_batch:
                    acc = pacc.tile([N, nb, M], F32, tag=f"acc{si % 2}")
                out_sb = outp.tile([N, nb, M], F32, tag=f"out{si % 2}")
                for k in range(nb):
                    if per_batch:
                        # own PSUM tile per batch: avoids the same-bank
                        # PE-write vs copy-read serialization within a pair
                        acc_k = pacc.tile([N, 1, M], F32, tag=f"accb{b0 + k}")
                        dst = acc_k[:, 0, :]
                    else:
                        dst = acc[:, k, :]
                    t, tk = tiles[b0 + k]
                    for c in range(KC):
                        mm = nc.tensor.matmul(
                            dst,
                            t[:, tk, c * N : (c + 1) * N],
                            t[:, tk, KC * N + c * M : KC * N + (c + 1) * M],
                            start=(c == 0),
                            stop=(c == KC - 1),
                        )
                        if pin_pe_order and prev_mm is not None:
                            # keep PE instruction order = batch order so PE
                            # never idles waiting on a later batch's load
                            tile.add_dep_helper(
                                mm.ins, prev_mm.ins, sync=False,
                                reason="pin PE batch order",
                            )
                        prev_mm = mm
                    if per_batch:
                        src = acc_k[:, 0, :]
                        if copy_eng[si] == "vector_b":
                            nc.vector.tensor_copy(out_sb[:, k, :], src)
                        elif copy_eng[si] == "scalar_b":
                            nc.scalar.copy(out_sb[:, k, :], src)
                        elif k % 2 == 0:  # split
                            nc.scalar.copy(out_sb[:, k, :], src)
                        else:
                            nc.vector.tensor_copy(out_sb[:, k, :], src)
                if not per_batch:
                    if copy_eng[si] == "vector":
                        nc.vector.tensor_copy(out_sb[:], acc[:])
                    else:
                        nc.scalar.copy(out_sb[:], acc[:])
                eng(store_eng[si]).dma_start(pg[:, b0 : b0 + nb, :], out_sb[:])
                b0 += nb

    if strip_init:
        _strip_init_barrier(nc, strip_sp_regs=strip_sp_regs)
    if strip_tail2:
        _strip_tail_barrier2(nc)

    _split_multi_waits(nc)
    return nc


BEST_CONFIG = dict(
    load_split=(2, 2),
    store_split=(2, 2),
    load_eng=("sync", "gpsimd"),
    store_eng=("scalar", "sync"),
    copy_eng=("split", "split"),
    in_dt=mybir.dt.bfloat16,
    strip_init=True,
    pin_pe_order=True,
    strip_sp_regs=True,
    strip_tail2=True,
)
HOST_DT = ml_dtypes.bfloat16

_NC_CACHE = None


def _get_nc():
    global _NC_CACHE
    if _NC_CACHE is None:
        _NC_CACHE = build_pg_kernel(**BEST_CONFIG)
    return _NC_CACHE


# --------------------------------------------------------------------------
# Host-side Hungarian (Jonker-Volgenant potentials form), matches
# scipy.linear_sum_assignment. Sequential and host-bound by nature.
def _hungarian(cost):
    cost = np.asarray(cost, dtype=np.float64)
    n, m = cost.shape
    u = np.zeros(n + 1)
    v = np.zeros(m + 1)
    p = np.zeros(m + 1, dtype=np.int64)
    way = np.zeros(m + 1, dtype=np.int64)
    for i in range(1, n + 1):
        p[0] = i
        j0 = 0
        minv = np.full(m + 1, np.inf)
        used = np.zeros(m + 1, dtype=bool)
        while True:
            used[j0] = True
            i0 = p[j0]
            free = ~used[1:]
            cur = cost[i0 - 1, :] - u[i0] - v[1:]
            upd = free & (cur < minv[1:])
            minv[1:][upd] = cur[upd]
            way[1:][upd] = j0
            masked = np.where(free, minv[1:], np.inf)
            j1 = int(np.argmin(masked)) + 1
            delta = masked[j1 - 1]
            u[p[used]] += delta
            v[used] -= delta
            minv[1:][free] -= delta
            j0 = j1
            if p[j0] == 0:
                break
        while j0:
            j1 = way[j0]
            p[j0] = p[j1]
            j0 = j1
    col_of_row = np.zeros(n, dtype=np.int64)
    for j in range(1, m + 1):
        if p[j] > 0:
            col_of_row[p[j] - 1] = j - 1
    return col_of_row


def _chunked_transpose(x):
    """[B, N, D] -> [B, 128, KC*N] with out[b, p, c*N+n] = x[b, n, c*128+p]."""
    # x.T per batch: [B, D, N] -> view as [B, KC, 128, N] -> [B, 128, KC, N]
    xt = x.transpose(0, 2, 1).reshape(x.shape[0], KC, 128, x.shape[1])
    return xt.transpose(0, 2, 1, 3).reshape(x.shape[0], 128, KC * x.shape[1])


def host_pack(p_full, g_full, dtype=np.float32):
    """Build the combined device input io [B, 128, XW]."""
    io = np.concatenate(
        [_chunked_transpose(p_full), _chunked_transpose(g_full)], axis=2
    )
    return np.ascontiguousarray(io.astype(dtype, copy=False))


def run_device_pg(p_full, g_full):
    """Shard over batch (d-major layout), run SPMD on 8 cores, gather pg."""
    nc = _get_nc()
    io = host_pack(p_full, g_full, dtype=HOST_DT)
    in_maps = [{"io": io[c * BPC : (c + 1) * BPC]} for c in range(N_CORES)]
    try:
        res = run_bass_kernel_spmd(nc, in_maps, list(range(N_CORES)))
    except ModuleNotFoundError:
        # BASS_TRACE set but the axon NTFF hook module isn't installed —
        # retry with tracing hard-disabled.
        import os

        os.environ["BASS_NEVER_TRACE"] = "1"
        res = run_bass_kernel_spmd(nc, in_maps, list(range(N_CORES)))
    # per-core result is pg_t[n, b, m] -> [b, n, m]
    return np.concatenate(
        [res.results[c]["pg"].transpose(1, 0, 2) for c in range(N_CORES)], axis=0
    )


def kernel(batch_prediction, batch_groundtruth):
    p_full = np.asarray(batch_prediction, dtype=np.float32)
    g_full = np.asarray(batch_groundtruth, dtype=np.float32)
    pg = run_device_pg(p_full, g_full)  # [B, N, M] f32 (bf16 products, f32 accum)

    # rank-1 terms + scaling on host (fp64), then Hungarian + matched mean
    p64 = p_full.astype(np.float64)
    g64 = g_full.astype(np.float64)
    p2 = np.einsum("bnd,bnd->bn", p64, p64)
    g2 = np.einsum("bmd,bmd->bm", g64, g64)
    cost = (p2[:, :, None] + g2[:, None, :] - 2.0 * pg.astype(np.float64)) / D

    total = 0.0
    rows = np.arange(N)
    for b in range(B):
        cols = _hungarian(cost[b])
        total += float(cost[b, rows, cols].mean())
    loss = np.float32(total / B * WEIGHT)
    return np.asarray([loss], dtype=np.float32)
